# revision 60
# baseline (speedup 1.0000x reference)
"""Trainium2 Bass kernel for DifferentiableRasterizer (point-to-mesh distance field).

out[b, n] = exp(-100 * min_f dist^2(points[b,n], tri[b,f]))

Strategy (8-core data-parallel, points axis sharded; tri_verts replicated):
  Host precomputes, per (batch, face), an orthonormal per-segment frame so the
  point-triangle distance decomposes into squares of AFFINE functionals of p:
     dist^2(p, seg_i) = ip_i(p)^2 + delta_i^2,  delta = max(sig-l, min(sig, 0))
     plane^2          = dnp(p)^2
     inside          <=> max_i ip_i(p) <= 0   (ip oriented outward)
     dist^2(p, tri)   = dnp^2 + (inside ? 0 : min_i dist2d_i)
  The 7 affine functionals per face (sig01,sig02,sig12, ip01,ip02,ip12, dnp)
  are computed on the TensorEngine as K=16 matmuls (two-limb bf16 split of
  homogeneous points), face-major (128 faces on partitions, points free).
  Face-chunks are processed in PAIRS; per pair:
    - PE: 14 matmuls (sig/ip 3-bank wide tiles per chunk; both dnp halves
      into one 2-bank tile, emitted last so a busy dnp bank never blocks
      the drain chain in PE program order);
    - ACT: two wide PSUM->SBUF drains per chunk (ips, sigs) - keeps every
      SEGSEL operand in SBUF (PSUM has one DVE read port, so one custom op
      can read at most one PSUM stream, and walrus rejects min/max
      TensorTensor on Pool, so elementwise combines cannot leave the DVE);
    - ACT drains the ips through Prelu(alpha=1e19): negative (gated) ip
      maps to -1e19*ip, so squaring sends gated lanes to ~inf WITHOUT a
      select in the DVE body - that frees the stage budget for PageIdx;
    - DVE per chunk: ONE paged dual-edge RAST_SEGU2 custom (edges 01+02,
      clamp bound steps s0 -> s0+s1 across pages; in1/out APs kept FLAT so
      the TTSS encoding retains both per-partition scalar slots) + ONE
      RAST_SEGU1 single (edge 12), strided pair-wide fp16 TT min-tree
      (2x rate), ONE double-width RAST_FINPLN custom (negated
      plane+inside-select over the dnp pair), double-width TT max
      accumulate;
    - tail (per point-chunk): fold the pair-halves (TT max), Pool
      partition_all_reduce(max) over the 128 face-slots (candidates are
      negated so the cross-partition reduce can be MAX - the GPSIMD ucode
      has no min), ACT exp(+ALPHA*x), DMA out.
  Warmup: a dummy matmul on a memset tile starts the PE p-state ramp at t~0,
  the first PAIR's SEGSELs read sig straight from PSUM (skipping the ss
  drain on the critical path; chunk 0 also drains ips per-edge), and gmat
  rides the Pool DMA queue (cheap ~25ns triggers; the ACT queue stays free
  for the drains that gate the SEGSELs).
  Engine busy (cost model, per core): DVE 203.5us (bottleneck, 93%),
  ACT 191us, PE 98us, Pool 20us; TimelineSim makespan ~218.7us vs 251.6us
  for the original structure. NOT possible (verified by neuronxcc compile
  probes, though TimelineSim happily prices them): min/max TensorTensor on
  Pool (ucode has add/mult only), ANY GPSIMD access to PSUM, matmul base
  partitions other than 0/32/64, select-GATED PageIdx SEGSELs (9 ALU
  stages > 8; the Prelu pre-gate is what makes the paged dual fit), and
  Lrelu's alpha is IGNORED (fixed 0.01 slope) - Prelu respects alpha.
"""

import numpy as np

B = 4
N = 8192
F = 1024
NCORES = 8
NP = N // NCORES          # points per core (per batch)
PC = 512                  # point-chunk (free dim)
NPC = NP // PC            # point chunks per batch
NFC = F // 128            # face chunks per batch
ALPHA = 100.0
MIN_TRI_AREA = 1e-5
BIGVAL = 1e18

# PE matmul dtype mode:
#   "fp32"   exact, 4 cyc/row
#   "fp32r"  1 cyc/row @ N>=256, ~tf32 precision (~5e-3 out err)
#   "bf16x4" 1 cyc/row, K=16 two-limb bf16 split per operand (~1e-4 out err)
MM_MODE = "bf16x4"
KDIM = 16 if MM_MODE == "bf16x4" else 4
USE_CUSTOM = True  # fused custom-DVE ops (7 DVE passes/chunk) vs stock ops
TRACE = False  # set True (before first kernel() call) to capture an NTFF profile

GATESLOPE = 1e19  # Prelu slope: sq() sends gated lanes to ~inf
BIGFILL = 1e30   # "not a candidate" fill for inside-masked segment distances
BIGTH = 1e29     # threshold detecting the fill


_DVE_OPS = {}


def _register_custom_ops():
    """Register the two fused DVE ops (idempotent)."""
    if _DVE_OPS:
        return _DVE_OPS
    from concourse.dve_spec import (
        Spec, Src0, Src1, C0, C1, C2, Zero, lower, maxx, minn, select, sq,
        PageIdx,
    )
    from concourse.dve_ops import DveOp, OPS, get_dve_sub_opcode, has_src1
    from concourse.dve_uop import DveOpSpec
    import numpy as _np

    import concourse.dve_ops as dve_ops_mod

    def _mk(name, spec, subdim=False):
        for op in OPS:
            if op.name == name:
                _DVE_OPS[name] = op
                return
        shas = {}
        op = DveOp(name, spec, subdim=subdim, uops_sha=shas)
        OPS.append(op)
        # the module builds these maps at import; extend them for new ops
        dve_ops_mod._SUB_OPCODE_FOR_NAME[name] = (
            dve_ops_mod._CUSTOM_DVE_ROW_BASE + len(OPS) - 1
        )
        dve_ops_mod.CUSTOM_DVE_SPECS[name] = spec
        for ver in ("v3", "v4"):
            s = DveOpSpec(
                name=name,
                opcode=get_dve_sub_opcode(name),
                uops=lower(spec, ver=ver),
                rd1_en=has_src1(spec),
            )
            shas[ver] = s.sha(ver)
        _DVE_OPS[name] = op

    # B = select(ip > 0, ip^2 + max(sig + s0, min(sig, 0))^2, BIGFILL)
    #   in0 = sig, in1 = ip, s0 = NEGATED segment length (per-partition),
    #   imm2 = BIGFILL
    _mk(
        "RAST_SEGSEL_ANT",
        Spec(
            body=select(
                Src1 > Zero,
                sq(Src1) + sq(maxx(Src0 + C0, minn(Src0, Zero))),
                C2,
            ),
            reference=lambda in0, in1, s0, imm2: _np.where(
                in1 > 0,
                in1 * in1
                + _np.square(_np.maximum(in0 + s0, _np.minimum(in0, 0.0))),
                imm2,
            ),
        ),
    )
    # fin = dnp^2 + (M >= BIGTH ? 0 : M);  in0 = M, in1 = dnp, s0 = BIGTH
    _mk(
        "RAST_FINPL_ANT",
        Spec(
            body=sq(Src1) + select(Src0 >= C0, Zero, Src0),
            reference=lambda in0, in1, s0: in1 * in1
            + _np.where(in0 >= s0, 0.0, in0),
        ),
    )
    # negating final: in0 = M (min of positive edge candidates; BIGFILL when
    # the point is inside all three half-planes), in1 = dnp, s0 = BIGTH.
    # fin = -(dnp^2 + (M >= BIGTH ? 0 : M))  ==  NEGATED dist^2 candidate,
    # so every downstream combine is MAX (cross-lane Pool reduce supports it).
    _mk(
        "RAST_FINPLN_ANT",
        Spec(
            body=select(Src0 >= C0, Zero, Zero - Src0) - sq(Src1),
            reference=lambda in0, in1, s0: _np.where(in0 >= s0, 0.0, -in0)
            - in1 * in1,
        ),
    )
    # UNGATED candidates (in1 is Prelu(alpha=1e19)-transformed ip: negative
    # ip maps to -1e19*ip, so sq() makes gated lanes huge/inf automatically -
    # no select needed, which frees the stage budget for PageIdx paging).
    # Dual: pages = edges (01, 02); clamp bound steps s0, s0+s1
    #       (s0 = -L01, s1 = L01 - L02).
    def _ref_segu2(in0, in1, s0, s1):
        P = in0.shape[0]
        a = in0.reshape(P, 2, in0.shape[-1])
        t = in1.reshape(a.shape).astype(_np.float64)
        s0 = _np.asarray(s0, _np.float64).reshape(P, 1, 1)
        s1 = _np.asarray(s1, _np.float64).reshape(P, 1, 1)
        pg = s0 + _np.arange(2, dtype=_np.float64).reshape(1, 2, 1) * s1
        u = _np.maximum(a + pg, _np.minimum(a, 0.0))
        return (t * t + u * u).reshape(in1.shape).astype(_np.float32)

    _mk(
        "RAST_SEGU2_ANT",
        Spec(
            body=sq(Src1)
            + sq(maxx(Src0 + PageIdx(C0, C1), minn(Src0, Zero))),
            reference=_ref_segu2,
        ),
        subdim=True,
    )
    _mk(
        "RAST_SEGU1_ANT",
        Spec(
            body=sq(Src1) + sq(maxx(Src0 + C0, minn(Src0, Zero))),
            reference=lambda in0, in1, s0: in1 * in1
            + _np.square(_np.maximum(in0 + s0, _np.minimum(in0, 0.0))),
        ),
    )
    return _DVE_OPS


def _host_face_constants(tri):
    """tri: (B, F, 3, 3) float32 -> per-face affine functional rows (float64)."""
    t = tri.astype(np.float64)
    v0, v1, v2 = t[:, :, 0, :], t[:, :, 1, :], t[:, :, 2, :]
    e0 = v1 - v0
    e1 = v2 - v0
    e12 = v2 - v1
    n = np.cross(e0, e1)
    area2 = (n * n).sum(-1)
    valid = area2 >= 4.0 * (MIN_TRI_AREA ** 2)
    nh = n / np.sqrt(np.maximum(area2, 1e-300))[..., None]

    def seg_const(a, d, opp):
        L = np.sqrt((d * d).sum(-1))
        eh = d / np.maximum(L, 1e-300)[..., None]
        m = np.cross(eh, nh)
        flip = (m * (opp - a)).sum(-1) > 0
        m = np.where(flip[..., None], -m, m)
        # sigma(p) = eh.p + eo ; ip(p) = m.p + mo
        return eh, -(eh * a).sum(-1), m, -(m * a).sum(-1), L

    segs = [seg_const(v0, e0, v2), seg_const(v0, e1, v1), seg_const(v1, e12, v0)]
    dn_c, dn_o = nh, -(nh * v0).sum(-1)

    inv = ~valid
    fixed = []
    for eh, eo, m, mo, L in segs:
        eh = np.where(inv[..., None], 0.0, eh)
        eo = np.where(inv, 0.0, eo)
        m = np.where(inv[..., None], 0.0, m)
        mo = np.where(inv, BIGVAL, mo)
        L = np.where(inv, 1.0, L)
        fixed.append((eh, eo, m, mo, L))
    dn_c = np.where(inv[..., None], 0.0, dn_c)
    dn_o = np.where(inv, BIGVAL, dn_o)
    return fixed, dn_c, dn_o, valid


def _host_pack(points, tri):
    """Build the DRAM input arrays for the device kernel."""
    segs, dn_c, dn_o, valid = _host_face_constants(tri)

    # G matrix: [B, NFC, 7, 4, 128]  (functional rows over homogeneous p)
    # functional order: sig01, sig02, sig12, ip01, ip02, ip12, dnp
    G = np.zeros((B, NFC, 7, 4, 128), np.float32)
    for k in range(3):
        eh, eo, m, mo, _ = segs[k]
        for b in range(B):
            gc = eh[b].reshape(NFC, 128, 3)
            go = eo[b].reshape(NFC, 128)
            G[b, :, k, 0:3, :] = gc.transpose(0, 2, 1)
            G[b, :, k, 3, :] = go
            ic = m[b].reshape(NFC, 128, 3)
            io = mo[b].reshape(NFC, 128)
            G[b, :, 3 + k, 0:3, :] = ic.transpose(0, 2, 1)
            G[b, :, 3 + k, 3, :] = io
    for b in range(B):
        nc_ = dn_c[b].reshape(NFC, 128, 3)
        no_ = dn_o[b].reshape(NFC, 128)
        G[b, :, 6, 0:3, :] = nc_.transpose(0, 2, 1)
        G[b, :, 6, 3, :] = no_

    # negated segment lengths for ACT bias: [128, B*NFC*3]
    lneg = np.zeros((128, B * NFC * 3), np.float32)
    for b in range(B):
        L0 = segs[0][4][b].reshape(NFC, 128)
        L1 = segs[1][4][b].reshape(NFC, 128)
        L2 = segs[2][4][b].reshape(NFC, 128)
        for fc in range(NFC):
            c = (b * NFC + fc) * 3
            lneg[:, c + 0] = -L0[fc]           # dual page0 clamp bound
            lneg[:, c + 1] = L0[fc] - L1[fc]   # dual page step (-> -L02)
            lneg[:, c + 2] = -L2[fc]           # single (edge 12)

    # homogeneous point tiles per core: [B, 4, NP] fp32
    pts_full = []
    for c in range(NCORES):
        ps = points[:, c * NP:(c + 1) * NP, :].astype(np.float32)  # (B, NP, 3)
        pt = np.ones((B, 4, NP), np.float32)
        pt[:, 0:3, :] = ps.transpose(0, 2, 1)
        pts_full.append(pt)

    if MM_MODE == "bf16x4":
        import ml_dtypes

        bf16 = ml_dtypes.bfloat16
        # two-limb bf16 split: x = hi + lo (+O(2^-18))
        Ghi = G.astype(bf16)
        Glo = (G - Ghi.astype(np.float32)).astype(bf16)
        # lhsT rows (K=16): [Ghi; Glo; Ghi; Glo]
        Gk = np.concatenate([Ghi, Glo, Ghi, Glo], axis=3)  # [B,NFC,7,16,128]
        Gk = np.ascontiguousarray(Gk.transpose(0, 3, 1, 2, 4)).reshape(
            B, KDIM, NFC * 7 * 128
        )
        ptils = []
        for pt in pts_full:
            phi = pt.astype(bf16)
            plo = (pt - phi.astype(np.float32)).astype(bf16)
            # rhs rows (K=16): [phi; phi; plo; plo]
            ptils.append(np.concatenate([phi, phi, plo, plo], axis=1))
        return Gk, lneg, ptils, valid

    # fp32/fp32r: K=4, pre-transpose for direct DMA (K on partitions)
    Gk = np.ascontiguousarray(G.transpose(0, 3, 1, 2, 4)).reshape(
        B, 4, NFC * 7 * 128
    )
    return Gk, lneg, pts_full, valid


def _host_invalid_min(points, tri, valid):
    """Exact min dist^2 over INVALID faces only (numpy, usually none)."""
    if valid.all():
        return None
    out = np.full((B, N), np.inf, np.float64)
    for b in range(B):
        idx = np.where(~valid[b])[0]
        if len(idx) == 0:
            continue
        t = tri[b, idx].astype(np.float64)   # (Fi, 3, 3)
        p = points[b].astype(np.float64)     # (N, 3)
        v0, v1, v2 = t[:, 0], t[:, 1], t[:, 2]

        def segd(a, d):
            L2 = np.maximum((d * d).sum(-1), 1e-12)
            tt = np.clip(((p[:, None, :] - a) * d).sum(-1) / L2, 0, 1)
            proj = a + tt[..., None] * d
            df = p[:, None, :] - proj
            return (df * df).sum(-1)

        dd = np.minimum(np.minimum(segd(v0, v1 - v0), segd(v0, v2 - v0)),
                        segd(v1, v2 - v1))
        out[b] = dd.min(-1)
    return out


def _build_bass(reps=1):
    import concourse.bass as bass
    import concourse.bacc as bacc
    import concourse.tile as tile
    from concourse import mybir

    f32 = mybir.dt.float32
    nc = bacc.Bacc(None)

    mmdt_in = {
        "fp32": f32,
        "fp32r": mybir.dt.float32r,
        "bf16x4": mybir.dt.bfloat16,
    }[MM_MODE]
    ptil = nc.declare_dram_parameter("ptil", [B, KDIM, NP], mmdt_in, isOutput=False)
    gmat = nc.declare_dram_parameter("gmat", [B, KDIM, NFC * 7 * 128], mmdt_in, isOutput=False)
    lneg = nc.declare_dram_parameter("lneg", [128, B * NFC * 3], f32, isOutput=False)
    outp = nc.declare_dram_parameter("out", [B, NP], f32, isOutput=True)

    mm_dt = f32 if MM_MODE == "fp32" else mybir.dt.float32r

    ACT = mybir.ActivationFunctionType
    ALU = mybir.AluOpType

    ops = _register_custom_ops()

    with tile.TileContext(nc) as tc:
        with (
            tc.tile_pool(name="const", bufs=1) as constp,
            tc.tile_pool(name="gp", bufs=2) as gpool,
            tc.tile_pool(name="pp", bufs=2) as ppool,
            tc.tile_pool(name="ps", bufs=1, space="PSUM") as psum,
            tc.tile_pool(name="psd", bufs=1, space="PSUM") as psum_d,
            tc.tile_pool(name="wk", bufs=3) as wk,
            tc.tile_pool(name="accp", bufs=2) as accp,
            tc.tile_pool(name="outs", bufs=2) as outsp,
        ):
            f16 = mybir.dt.float16
            # early dummy matmul on a memset tile: starts the PE p-state
            # ramp immediately (no DMA dependency) so the first real matmuls
            # run at full clock
            w0 = constp.tile([KDIM, 640], f32, tag="w0")
            nc.gpsimd.memset(w0[:], 0)
            warm = psum_d.tile([128, 2 * PC], f32, tag="ps_dnp", name="warm")
            nc.tensor.matmul(
                warm[:, 0:PC], w0[:, 0:128], w0[:, 128:640],
                start=True, stop=True,
            )
            ltile = constp.tile([128, B * NFC * 3], f32, tag="lneg")
            nc.sync.dma_start(ltile[:], lneg[:])

            def emit_tail(b, pc, acc, split=1):
                # fold the chunk-pair halves, then: acc holds NEGATED
                # running-max candidates (FINPLN), so the face-slot reduce is
                # a Pool partition_all_reduce(max); then exp(ALPHA * -d^2).
                # The very last tail runs as two half-width chains so the
                # end-of-kernel serial fold->reduce->exp->DMA chain shortens.
                import concourse.bass_isa as bass_isa
                w = PC // split
                for s in range(split):
                    sl = slice(s * w, s * w + w)
                    accf = outsp.tile([128, w], f16, tag="accf", name="accf")
                    nc.vector.tensor_tensor(
                        accf[:], acc[:, s * w:s * w + w],
                        acc[:, PC + s * w:PC + s * w + w], op=ALU.max)
                    dmax = outsp.tile([128, w], f32, tag="dmax", name="dmax")
                    nc.gpsimd.partition_all_reduce(
                        dmax[:], accf[:], channels=128,
                        reduce_op=bass_isa.ReduceOp.max,
                    )
                    eo = outsp.tile([1, w], f32, tag="eo", name="eo")
                    nc.scalar.activation(eo[:], dmax[0:1, :], ACT.Exp, scale=ALPHA)
                    nc.sync.dma_start(
                        outp[b, pc * PC + s * w:pc * PC + s * w + w], eo[:])

            for rep in range(reps):
              for b in range(B):
                # gmat split into quarters over the ACT and Pool DMA queues
                # (parallel transfers off the SP queue that carries pt), so
                # the first face-chunks' weights land early
                # all gmat quarters ride the Pool DMA queue: its trigger cost
                # is ~25ns vs ~650ns on SP/ACT, and it must not displace the
                # ACT drains that gate the first SEGSELs
                pt = ppool.tile([KDIM, NP], mmdt_in, tag="p")
                nc.sync.dma_start(pt[:], ptil[b])
                gt = gpool.tile([KDIM, NFC * 7 * 128], mmdt_in, tag="g")
                u = 7 * 128
                nc.gpsimd.dma_start(gt[:, 0:u], gmat[b, :, 0:u])
                nc.gpsimd.dma_start(gt[:, u:4 * u], gmat[b, :, u:4 * u])
                nc.gpsimd.dma_start(gt[:, 4 * u:], gmat[b, :, 4 * u:])

                for pc in range(NPC):
                    rhs = pt[:, pc * PC:(pc + 1) * PC]
                    # acc holds a PAIR of face-chunk accumulators side by
                    # side; they are folded once at the tail
                    acc = accp.tile([128, 2 * PC], f16, tag="acc")

                    # face-chunks processed in PAIRS so the min-tree and the
                    # accumulate run as double-width fp16 TTs (amortizes the
                    # per-op SBUF access + instruction overhead)
                    for pp in range(NFC // 2):
                        # Dq: [D01(h0) | D02(h0) | D01(h1) | D02(h1)]
                        Dq = wk.tile([128, 4 * PC], f16, tag="Dq")
                        S12p = wk.tile([128, 2 * PC], f16, tag="S12p")
                        # dnp matmuls for BOTH halves go into one 2-bank tile
                        # consumed by a single double-width FINPLN; emitted
                        # after the sig/ip matmuls so a stalled dnp bank never
                        # blocks the drain chain in PE program order
                        dnpw = psum_d.tile([128, 2 * PC], f32, tag="ps_dnp")
                        for half in range(2):
                            fc = 2 * pp + half
                            # --- PE: sig/ip functionals -> PSUM ---
                            sgs = psum.tile([128, 3 * PC], f32, tag="ps_sig")
                            for phi in range(3):
                                lhsT = gt[:, (fc * 7 + phi) * 128:(fc * 7 + phi + 1) * 128]
                                nc.tensor.matmul(
                                    sgs[:, phi * PC:(phi + 1) * PC], lhsT, rhs,
                                    start=True, stop=True,
                                )
                            ips = psum.tile([128, 3 * PC], f32, tag="ps_ip")
                            for k in range(3):
                                lhsT = gt[:, (fc * 7 + 3 + k) * 128:(fc * 7 + 4 + k) * 128]
                                nc.tensor.matmul(
                                    ips[:, k * PC:(k + 1) * PC], lhsT, rhs,
                                    start=True, stop=True,
                                )
                            lhsT = gt[:, (fc * 7 + 6) * 128:(fc * 7 + 7) * 128]
                            nc.tensor.matmul(
                                dnpw[:, half * PC:(half + 1) * PC], lhsT, rhs,
                                start=True, stop=True,
                            )

                            # --- ACT: wide PSUM->SBUF drains ---
                            # The very first chunk skips the sig drain
                            # entirely (its SEGSELs read sig from PSUM, one
                            # PSUM stream per op is legal) and drains ips
                            # per-edge, so the first SEGSEL starts as soon as
                            # one narrow copy lands.
                            first = (rep == 0 and b == 0 and pc == 0 and fc <= 1)
                            ics = wk.tile([128, 3 * PC], f32, tag="ics")
                            if first:
                                ss = None
                                if fc == 0:
                                    s0_, s12 = slice(0, 2 * PC), slice(2 * PC, 3 * PC)
                                    nc.scalar.activation(
                                        ics[:, s0_], ips[:, s0_], ACT.Prelu,
                                        alpha=GATESLOPE)
                                    nc.scalar.activation(
                                        ics[:, s12], ips[:, s12], ACT.Prelu,
                                        alpha=GATESLOPE)
                                else:
                                    nc.scalar.activation(
                                        ics[:], ips[:], ACT.Prelu,
                                        alpha=GATESLOPE)
                            else:
                                ss = wk.tile([128, 3 * PC], f32, tag="ss")
                                nc.scalar.activation(
                                    ics[:], ips[:], ACT.Prelu, alpha=GATESLOPE)
                                nc.scalar.activation(ss[:], sgs[:], ACT.Copy)

                            # --- DVE: fused UNGATED candidates (fp16) ---
                            # dual op: edges (01, 02) as PageIdx pages; the
                            # in1/out APs stay FLAT (same element order) so
                            # the TTSS encoding keeps both scalar slots
                            base = (b * NFC + fc) * 3
                            srcsig = sgs if first else ss
                            in0d = srcsig[:, 0:2 * PC].rearrange(
                                "p (s n) -> p s n", s=2)
                            nc.vector._custom_dve(
                                ops["RAST_SEGU2_ANT"],
                                out=Dq[:, half * 2 * PC:(half + 1) * 2 * PC],
                                in0=in0d,
                                in1=ics[:, 0:2 * PC],
                                s0=ltile[:, base:base + 1],
                                s1=ltile[:, base + 1:base + 2],
                            )
                            nc.vector._custom_dve(
                                ops["RAST_SEGU1_ANT"],
                                out=S12p[:, half * PC:(half + 1) * PC],
                                in0=srcsig[:, 2 * PC:3 * PC],
                                in1=ics[:, 2 * PC:3 * PC],
                                s0=ltile[:, base + 2:base + 3],
                            )

                        # --- DVE: double-width min-tree + plane/inside ---
                        dv = Dq[:].rearrange("p (a c n) -> p a c n", a=2, c=2)
                        m1 = wk.tile([128, 2 * PC], f16, tag="m1")
                        nc.vector.tensor_tensor(
                            m1[:], dv[:, :, 0, :], dv[:, :, 1, :], op=ALU.min)
                        m2 = wk.tile([128, 2 * PC], f16, tag="m2")
                        nc.vector.tensor_tensor(m2[:], m1[:], S12p[:], op=ALU.min)
                        if pp == 0:
                            nc.vector._custom_dve(
                                ops["RAST_FINPLN_ANT"],
                                out=acc[:], in0=m2[:], in1=dnpw[:], s0=BIGTH,
                            )
                        else:
                            finw = wk.tile([128, 2 * PC], f16, tag="finw")
                            nc.vector._custom_dve(
                                ops["RAST_FINPLN_ANT"],
                                out=finw[:], in0=m2[:], in1=dnpw[:], s0=BIGTH,
                            )
                            nc.vector.tensor_tensor(
                                acc[:], acc[:], finw[:], op=ALU.max)

                    last = (rep == reps - 1 and b == B - 1 and pc == NPC - 1)
                    emit_tail(b, pc, acc, split=2 if last else 1)

    nc.finalize()
    return nc


_CACHED = {}


def kernel(points: np.ndarray, tri_verts: np.ndarray) -> np.ndarray:
    points = np.asarray(points)
    tri_verts = np.asarray(tri_verts)
    assert points.shape == (B, N, 3) and tri_verts.shape == (B, F, 3, 3)

    from concourse.bass_utils import run_bass_kernel_spmd

    G, lneg, ptils, valid = _host_pack(points, tri_verts)

    if "nc" not in _CACHED:
        _CACHED["nc"] = _build_bass()
    nc = _CACHED["nc"]

    in_maps = [
        {"ptil": ptils[c], "gmat": G, "lneg": lneg}
        for c in range(NCORES)
    ]
    res = run_bass_kernel_spmd(
        nc, in_maps, core_ids=list(range(NCORES)), trace=TRACE
    )
    _CACHED["last_exec_ns"] = res.exec_time_ns
    _CACHED["last_res"] = res

    out = np.zeros((B, N), np.float32)
    for c in range(NCORES):
        out[:, c * NP:(c + 1) * NP] = res.results[c]["out"]

    inv_min = _host_invalid_min(points, tri_verts, valid)
    if inv_min is not None:
        out = np.maximum(out, np.exp(-ALPHA * inv_min).astype(np.float32))
    return out

